# revision 1
# baseline (speedup 1.0000x reference)
"""CaptioningRNN forward loss on 8 Trainium2 NeuronCores.

Strategy:
  - The LSTM recurrence is replicated on all 8 cores (per-step PE cost is
    bound by streaming Wh through the PE array regardless of batch size, so
    data-parallel would save nothing and force cross-core exchanges).
  - The large output projection h @ W_vocab (4096 x 512 x 32000) is sharded
    over the vocab axis: each core holds 4000 columns (as fp8 * 64, matmul'd
    with fp8 DoubleRow at 2x rate) and computes, per (row, t), the partial
    sum_v exp(logit_v) fused on-chip (ACT exp(x/64) with accumulate) so
    logits never hit HBM. fp8 noise is per-column independent and averages
    out in the 4000-term exp-sum; the target logit is computed exactly on
    every core as a DVE fp32 dot product h_t . W_vocab[:, target] using
    host-gathered target columns.
  - Gates are computed in two 1024-column slices (columns permuted to
    [i_j|f_j|o_j|g_j] blocks of 256) so the ACT/DVE gate chain of slice 0
    overlaps the PE matmuls of slice 1; h-transposes write back into each
    slice's already-consumed g-region of PSUM.
  - LSTM matmuls optionally also run as fp8 DoubleRow (split scales:
    x*16 @ Wx*4 and h @ Wh*64, both accumulating 64*A; the 1/64 descale is
    folded into the ACT sigmoid/tanh `scale`); fall back to float32r.
  - Host combines: loss = sum(mask * (log(sum_cores S) - tgt)) / N.
"""

import numpy as np
import ml_dtypes

import concourse.bass as bass
import concourse.tile as tile
from concourse import mybir, bacc
from concourse.bass_utils import run_bass_kernel_spmd

F32 = mybir.dt.float32
F32R = mybir.dt.float32r
FP8 = mybir.dt.float8e4

# Problem shape (hardcoded per task spec)
N = 128          # batch
T1 = 32          # caption steps (T-1)
D_FEAT = 1280
W_DIM = 256
H = 512
V = 32000
NCORES = 8
VS = V // NCORES          # 4000 vocab cols per core
NSL = 4                   # vocab slices per core (1000 cols each)
SL = VS // NSL            # 1000 cols per slice (2 x 500 halves)
WV_SCALE = 64.0           # W_vocab fp8 scale (descaled in ACT exp)
X_SCALE = 16.0            # x_t fp8 scale   (LSTM fp8 path)
WX_SCALE = 4.0            # Wx fp8 scale    (X_SCALE*WX_SCALE == WV_SCALE)
LSTM_FP8 = True
NULL = 0

_CACHE = {}


def _build(zero_b, zero_bp, zero_bv, repeats=1, lstm_fp8=LSTM_FP8):
    nc = bacc.Bacc("TRN2", target_bir_lowering=False, debug=False)

    if lstm_fp8:
        xt_d = nc.dram_tensor("xt8", [T1, 128, 2, 128], FP8,
                              kind="ExternalInput")
        wb_d = nc.dram_tensor("wb8", [128, 6, 4 * H], FP8,
                              kind="ExternalInput")
    else:
        xt_d = nc.dram_tensor("xt", [T1, 2, 128, 128], F32R,
                              kind="ExternalInput")
        wb_d = nc.dram_tensor("wb", [6, 128, 4 * H], F32R,
                              kind="ExternalInput")
    ft_d = nc.dram_tensor("ft", [10, 128, 128], F32R, kind="ExternalInput")
    wp_d = nc.dram_tensor("wp", [10, 128, H], F32R, kind="ExternalInput")
    wv_d = nc.dram_tensor("wv8", [128, 4, VS], FP8, kind="ExternalInput")
    wt_d = nc.dram_tensor("wt", [T1, 128, H], F32, kind="ExternalInput")
    id_d = nc.dram_tensor("ident", [128, 128], F32, kind="ExternalInput")
    if not (zero_b and zero_bp):
        ones_d = nc.dram_tensor("ones", [1, 128], F32R, kind="ExternalInput")
    if not zero_b:
        bvec_d = nc.dram_tensor("bvec", [1, 4 * H], F32R, kind="ExternalInput")
    if not zero_bp:
        bp_d = nc.dram_tensor("bp", [1, H], F32R, kind="ExternalInput")
    if not zero_bv:
        ebv_d = nc.dram_tensor("ebv", [1, VS], F32, kind="ExternalInput")
        bt_d = nc.dram_tensor("bt", [128, T1], F32, kind="ExternalInput")
    s_d = nc.dram_tensor("S_out", [128, T1], F32, kind="ExternalOutput")
    tgt_d = nc.dram_tensor("tgt_out", [128, T1], F32, kind="ExternalOutput")

    GSCALE = (1.0 / WV_SCALE) if lstm_fp8 else 1.0
    AF = mybir.ActivationFunctionType
    DR = mybir.MatmulPerfMode.DoubleRow
    with tile.TileContext(nc) as tc:
        with tc.tile_pool(name="const", bufs=1) as constp, \
             tc.tile_pool(name="wbp", bufs=1) as wbp, \
             tc.tile_pool(name="xk", bufs=3) as xkp, \
             tc.tile_pool(name="wtp", bufs=3) as wtp, \
             tc.tile_pool(name="hpool", bufs=3) as hp, \
             tc.tile_pool(name="gates", bufs=3) as gp, \
             tc.tile_pool(name="scr", bufs=4) as scrp, \
             tc.tile_pool(name="h0w", bufs=3) as h0w, \
             tc.tile_pool(name="psA", bufs=2, space="PSUM") as psA, \
             tc.tile_pool(name="psV", bufs=2, space="PSUM") as psV:

            # --- constants / resident weights -----------------------------
            ident = constp.tile([128, 128], F32, tag="ident")
            nc.sync.dma_start(out=ident, in_=id_d[:, :])
            if lstm_fp8:
                wb8 = constp.tile([128, 6, 4 * H], FP8, tag="wb8")
                nc.sync.dma_start(out=wb8, in_=wb_d[:, :, :])
            else:
                wb_sb = []
                for k in range(6):
                    wbt = wbp.tile([128, 4 * H], F32R, tag=f"wb{k}")
                    nc.sync.dma_start(out=wbt, in_=wb_d[k])
                    wb_sb.append(wbt)
            wv8 = constp.tile([128, 4, VS], FP8, tag="wv8")
            nc.sync.dma_start(out=wv8, in_=wv_d[:, :, :])
            if not (zero_b and zero_bp):
                ones_sb = constp.tile([1, 128], F32R, tag="ones")
                nc.sync.dma_start(out=ones_sb, in_=ones_d[:, :])
            if not zero_b:
                bvec_sb = constp.tile([1, 4 * H], F32R, tag="bvec")
                nc.sync.dma_start(out=bvec_sb, in_=bvec_d[:, :])
            if not zero_bp:
                bp_sb = constp.tile([1, H], F32R, tag="bp")
                nc.sync.dma_start(out=bp_sb, in_=bp_d[:, :])
            if not zero_bv:
                ebv_sb = constp.tile([128, VS], F32, tag="ebv")
                nc.sync.dma_start(
                    out=ebv_sb,
                    in_=bass.AP(tensor=ebv_d, offset=0, ap=[[0, 128], [1, VS]]))
                bt_sb = constp.tile([128, T1], F32, tag="bt")
                nc.sync.dma_start(out=bt_sb, in_=bt_d[:, :])

            # persistent state
            c_t = constp.tile([128, H], F32, tag="c")
            S_acc = constp.tile([128, T1], F32, tag="Sacc")
            tgt_acc = constp.tile([128, T1], F32, tag="tgtacc")

            for _rep in range(repeats):
                nc.vector.memset(c_t, 0.0)

                def emit_vocab(hT8, vs, Ssl):
                    h8v = hT8.rearrange("p (j m) -> p j m", j=4)
                    pV = psV.tile([128, 2, 512], F32, tag="pV")
                    for k in range(2):
                        for hh in range(2):
                            nc.tensor.matmul(
                                pV[:, hh, 0:500],
                                h8v[:, 2 * k:2 * k + 2, :],
                                wv8[:, 2 * k:2 * k + 2,
                                    vs * SL + hh * 500:
                                    vs * SL + (hh + 1) * 500],
                                start=(k == 0), stop=(k == 1),
                                perf_mode=DR)
                    ex = scrp.tile([128, 2, 500], F32, tag="ex")
                    if zero_bv:
                        nc.scalar.activation(
                            ex, pV[:, :, 0:500], AF.Exp, scale=1.0 / WV_SCALE,
                            accum_out=Ssl[:, vs:vs + 1])
                    else:
                        nc.scalar.activation(
                            ex, pV[:, :, 0:500], AF.Exp, scale=1.0 / WV_SCALE)
                        exw = scrp.tile([128, 2, 500], F32, tag="exw")
                        ebv_v = ebv_sb.rearrange("p (s hh m) -> p s hh m",
                                                 s=NSL, hh=2)
                        nc.vector.tensor_mul(exw, ex, ebv_v[:, vs])
                        nc.vector.tensor_reduce(
                            out=Ssl[:, vs:vs + 1], in_=exw,
                            axis=mybir.AxisListType.XY, op=mybir.AluOpType.add)

                def lstm_slice_mms(A_j, j, xk, hT_lhs):
                    # A_j [128, 1024] accumulates slice j of the (permuted)
                    # gate pre-activations for [x_t | h] @ [Wx; Wh]
                    nbias = 0 if zero_b else 1
                    if lstm_fp8:
                        xv = xk  # [128, 2, 128] fp8 pair
                        hv = hT_lhs.rearrange("p (j m) -> p j m", j=4)
                        pairs = [xv, hv[:, 0:2, :], hv[:, 2:4, :]]
                        for k in range(3):
                            for hh in range(2):
                                nc.tensor.matmul(
                                    A_j[:, hh * H:(hh + 1) * H], pairs[k],
                                    wb8[:, 2 * k:2 * k + 2,
                                        j * 1024 + hh * H:
                                        j * 1024 + (hh + 1) * H],
                                    start=(k == 0),
                                    stop=(k == 2 and nbias == 0),
                                    perf_mode=DR)
                    else:
                        lhs = [xk[0], xk[1],
                               hT_lhs[:, 0:128], hT_lhs[:, 128:256],
                               hT_lhs[:, 256:384], hT_lhs[:, 384:512]]
                        for k in range(6):
                            nc.tensor.matmul(
                                A_j, lhs[k],
                                wb_sb[k][:, j * 1024:(j + 1) * 1024],
                                start=(k == 0),
                                stop=(k == 5 and nbias == 0))
                    if not zero_b:
                        nc.tensor.matmul(
                            A_j, ones_sb,
                            bvec_sb[:, j * 1024:(j + 1) * 1024],
                            start=False, stop=True)

                def gate_slice(A_j, j, h_new):
                    # A_j = [i_j | f_j | o_j | g_j] (256 each, pre-scaled)
                    blk = slice(j * 256, (j + 1) * 256)
                    sig = gp.tile([128, 768], F32, tag="sig")
                    nc.scalar.activation(sig, A_j[:, 0:768], AF.Sigmoid,
                                         scale=GSCALE)
                    g_g = gp.tile([128, 256], F32, tag="gg")
                    nc.scalar.activation(g_g, A_j[:, 768:1024], AF.Tanh,
                                         scale=GSCALE)
                    ig = gp.tile([128, 256], F32, tag="ig")
                    nc.vector.tensor_mul(ig, sig[:, 0:256], g_g)
                    fc = gp.tile([128, 256], F32, tag="fc")
                    nc.vector.tensor_mul(fc, sig[:, 256:512], c_t[:, blk])
                    nc.vector.tensor_add(c_t[:, blk], ig, fc)
                    tc_ = gp.tile([128, 256], F32, tag="tc")
                    nc.scalar.activation(tc_, c_t[:, blk], AF.Tanh)
                    nc.vector.tensor_mul(h_new[:, blk], sig[:, 512:768], tc_)
                    return tc_

                def transp_slice(A_j, j, h_new, hT8, hT):
                    # h.T blocks (2j, 2j+1) into A_j's consumed g-region,
                    # then cast out to fp8 (and f32r when needed)
                    for b in range(2):
                        nc.tensor.transpose(
                            A_j[:, 768 + b * 128:768 + (b + 1) * 128],
                            h_new[:, j * 256 + b * 128:
                                  j * 256 + (b + 1) * 128], ident)
                    nc.vector.tensor_copy(
                        hT8[:, j * 256:(j + 1) * 256], A_j[:, 768:1024])
                    if hT is not None:
                        nc.vector.tensor_copy(
                            hT[:, j * 256:(j + 1) * 256], A_j[:, 768:1024])

                # --- h0 = features @ W_proj (+ b_proj) ---------------------
                A0 = psA.tile([128, 1024], F32, tag="A")
                nmm = 10 if zero_bp else 11
                for k in range(10):
                    fk = h0w.tile([128, 128], F32R, tag="fk")
                    nc.sync.dma_start(out=fk, in_=ft_d[k])
                    wpk = h0w.tile([128, H], F32R, tag="wpk")
                    nc.sync.dma_start(out=wpk, in_=wp_d[k])
                    nc.tensor.matmul(A0[:, 0:H], fk, wpk,
                                     start=(k == 0), stop=(k == nmm - 1))
                if not zero_bp:
                    nc.tensor.matmul(A0[:, 0:H], ones_sb, bp_sb,
                                     start=False, stop=True)
                h_sb = hp.tile([128, H], F32, tag="h")
                nc.vector.tensor_copy(h_sb, A0[:, 0:H])
                hT8_prev = hp.tile([128, H], FP8, tag="hT8")
                hT_prev = (None if lstm_fp8 else
                           hp.tile([128, H], F32R, tag="hT"))
                for b in range(4):
                    nc.tensor.transpose(
                        A0[:, 512 + b * 128:512 + (b + 1) * 128],
                        h_sb[:, b * 128:(b + 1) * 128], ident)
                nc.vector.tensor_copy(hT8_prev, A0[:, 512:1024])
                if hT_prev is not None:
                    nc.vector.tensor_copy(hT_prev, A0[:, 512:1024])
                Ssl_prev = None

                for t in range(T1):
                    if lstm_fp8:
                        xk = xkp.tile([128, 2, 128], FP8, tag="xk")
                        nc.sync.dma_start(out=xk, in_=xt_d[t])
                    else:
                        xk0 = xkp.tile([128, 128], F32R, tag="xk0")
                        nc.sync.dma_start(out=xk0, in_=xt_d[t, 0])
                        xk1 = xkp.tile([128, 128], F32R, tag="xk1")
                        nc.sync.dma_start(out=xk1, in_=xt_d[t, 1])
                        xk = (xk0, xk1)
                    wt_t = wtp.tile([128, H], F32, tag="wt")
                    nc.sync.dma_start(out=wt_t, in_=wt_d[t])

                    hT_lhs = hT8_prev if lstm_fp8 else hT_prev
                    A_0 = psA.tile([128, 1024], F32, tag="A")
                    lstm_slice_mms(A_0, 0, xk, hT_lhs)
                    A_1 = psA.tile([128, 1024], F32, tag="A")
                    lstm_slice_mms(A_1, 1, xk, hT_lhs)

                    h_new = hp.tile([128, H], F32, tag="h")
                    hT8_new = hp.tile([128, H], FP8, tag="hT8")
                    hT_new = (None if lstm_fp8 else
                              hp.tile([128, H], F32R, tag="hT"))

                    gate_slice(A_0, 0, h_new)
                    if t >= 1:
                        emit_vocab(hT8_prev, 0, Ssl_prev)
                    gate_slice(A_1, 1, h_new)
                    transp_slice(A_0, 0, h_new, hT8_new, hT_new)
                    if t >= 1:
                        emit_vocab(hT8_prev, 1, Ssl_prev)
                    transp_slice(A_1, 1, h_new, hT8_new, hT_new)
                    if t >= 1:
                        emit_vocab(hT8_prev, 2, Ssl_prev)
                        emit_vocab(hT8_prev, 3, Ssl_prev)
                        nc.vector.tensor_reduce(
                            out=S_acc[:, t - 1:t], in_=Ssl_prev,
                            axis=mybir.AxisListType.X, op=mybir.AluOpType.add)

                    # target logit: tgt[n] = h_new[n, :] . wt_t[n, :] (+ bt)
                    prod = scrp.tile([128, H], F32, tag="prod")
                    nc.vector.tensor_mul(prod, h_new, wt_t)
                    if zero_bv:
                        nc.vector.tensor_reduce(
                            out=tgt_acc[:, t:t + 1], in_=prod,
                            axis=mybir.AxisListType.X, op=mybir.AluOpType.add)
                    else:
                        tred = scrp.tile([128, 1], F32, tag="tred")
                        nc.vector.tensor_reduce(
                            out=tred, in_=prod,
                            axis=mybir.AxisListType.X, op=mybir.AluOpType.add)
                        nc.vector.tensor_add(
                            tgt_acc[:, t:t + 1], tred, bt_sb[:, t:t + 1])

                    Ssl_prev = scrp.tile([128, NSL], F32, tag="Ssl")
                    hT8_prev, hT_prev = hT8_new, hT_new

                for vs in range(NSL):
                    emit_vocab(hT8_prev, vs, Ssl_prev)
                nc.vector.tensor_reduce(
                    out=S_acc[:, T1 - 1:T1], in_=Ssl_prev,
                    axis=mybir.AxisListType.X, op=mybir.AluOpType.add)

            nc.sync.dma_start(out=s_d[:, :], in_=S_acc)
            nc.sync.dma_start(out=tgt_d[:, :], in_=tgt_acc)

    nc.finalize()
    return nc


def _gate_perm():
    # slice j (1024 cols) = [i_j | f_j | o_j | g_j], 256-col blocks
    return np.concatenate([
        np.arange(base + j * 256, base + (j + 1) * 256)
        for j in range(2) for base in (0, H, 2 * H, 3 * H)])


def _prep_inputs(features, captions, W_proj, b_proj, W_embed, Wx, Wh, b,
                 W_vocab, b_vocab, lstm_fp8=LSTM_FP8):
    features = np.asarray(features, dtype=np.float32)
    captions = np.asarray(captions)
    W_proj = np.asarray(W_proj, dtype=np.float32)
    b_proj = np.asarray(b_proj, dtype=np.float32)
    W_embed = np.asarray(W_embed, dtype=np.float32)
    Wx = np.asarray(Wx, dtype=np.float32)
    Wh = np.asarray(Wh, dtype=np.float32)
    b = np.asarray(b, dtype=np.float32)
    W_vocab = np.asarray(W_vocab, dtype=np.float32)
    b_vocab = np.asarray(b_vocab, dtype=np.float32)

    captions_in = captions[:, :-1].astype(np.int64)
    captions_out = captions[:, 1:].astype(np.int64)

    zero_b = bool(np.all(b == 0))
    zero_bp = bool(np.all(b_proj == 0))
    zero_bv = bool(np.all(b_vocab == 0))

    perm = _gate_perm()
    x_emb = W_embed[captions_in]                            # [128, 32, 256]
    ft = np.ascontiguousarray(features.T.reshape(10, 128, 128))
    wp = np.ascontiguousarray(W_proj.reshape(10, 128, H))
    # gathered target columns: wt[t, n, :] = W_vocab[:, captions_out[n, t]]
    wt = np.ascontiguousarray(
        W_vocab.T[captions_out].transpose(1, 0, 2))         # [32, 128, 512]
    ident = np.eye(128, dtype=np.float32)

    common = {"ft": ft, "wp": wp, "wt": wt, "ident": ident}
    if lstm_fp8:
        Wb = np.concatenate([Wx * WX_SCALE, Wh * WV_SCALE], axis=0)[:, perm]
        common["wb8"] = np.ascontiguousarray(
            Wb.reshape(6, 128, 4 * H).transpose(1, 0, 2)
        ).astype(ml_dtypes.float8_e4m3)
        common["xt8"] = np.ascontiguousarray(
            (x_emb * X_SCALE).transpose(1, 2, 0)
            .reshape(T1, 2, 128, 128).transpose(0, 2, 1, 3)
        ).astype(ml_dtypes.float8_e4m3)
    else:
        Wb = np.concatenate([Wx, Wh], axis=0)[:, perm]
        common["wb"] = np.ascontiguousarray(Wb.reshape(6, 128, 4 * H))
        common["xt"] = np.ascontiguousarray(
            x_emb.transpose(1, 2, 0).reshape(T1, 2, 128, 128))
    if not (zero_b and zero_bp):
        common["ones"] = np.ones((1, 128), dtype=np.float32)
    if not zero_b:
        bscale = WV_SCALE if lstm_fp8 else 1.0
        common["bvec"] = (b[perm] * bscale).reshape(1, 4 * H)
    if not zero_bp:
        common["bp"] = b_proj.reshape(1, H)
    if not zero_bv:
        common["bt"] = np.ascontiguousarray(
            b_vocab[captions_out].astype(np.float32))       # [128, 32]

    in_maps = []
    for c in range(NCORES):
        m = dict(common)
        wv_shard = (W_vocab[:, c * VS:(c + 1) * VS] * WV_SCALE)
        m["wv8"] = np.ascontiguousarray(
            wv_shard.reshape(4, 128, VS).transpose(1, 0, 2)
        ).astype(ml_dtypes.float8_e4m3)
        if not zero_bv:
            m["ebv"] = np.exp(
                b_vocab[c * VS:(c + 1) * VS]).reshape(1, VS).astype(np.float32)
        in_maps.append(m)
    return in_maps, captions_out, (zero_b, zero_bp, zero_bv)


def kernel(features, captions, W_proj, b_proj, W_embed, Wx, Wh, b,
           W_vocab, b_vocab):
    in_maps, captions_out, key = _prep_inputs(
        features, captions, W_proj, b_proj, W_embed, Wx, Wh, b,
        W_vocab, b_vocab)
    if key not in _CACHE:
        _CACHE[key] = _build(*key)
    nc = _CACHE[key]

    res = run_bass_kernel_spmd(nc, in_maps, core_ids=list(range(NCORES)))
    global last_results
    last_results = res

    S_total = np.zeros((128, T1), dtype=np.float64)
    for c in range(NCORES):
        S_total += res.results[c]["S_out"].astype(np.float64)
    tgt = res.results[0]["tgt_out"].astype(np.float64)      # [128, 32]
    lse = np.log(S_total)
    mask = (captions_out != NULL)
    loss = (np.where(mask, lse - tgt, 0.0)).sum() / N
    return np.float32(loss)

